# revision 1
# baseline (speedup 1.0000x reference)
"""Trainium2 Bass kernel for nn_GCNNet (3-layer GCNConv+BN+ReLU, JK concat),
distributed over 8 NeuronCores.

Strategy (graph parallel): nodes are partitioned across the 8 cores
(round-robin by degree, then bin-packed into 128-node tiles balancing
per-tile in-edge counts).  Each edge is assigned to the core that owns its
destination node.  Per layer:
  m = (o @ W_l) * dinv          computed node-sharded, feature-major o
  all-gather of m (AllGather collective -> per-core DRAM replica)
  raw_agg[d] = sum_{e:dst=d} m[src_e]   via dma_gather (128 rows/chunk) and
                                        a one-hot selection matmul on PE that
                                        performs the segmented sum in PSUM
  agg = raw_agg * dinv[dst]     (GCNConv bias cancels inside BatchNorm)
  BN stats via a tiny AllReduce; o' = relu(a*agg + b) on the scalar engine.
The JK concat's first block is x itself (host-side copy); per-layer outputs
return feature-major and are transposed/unpermuted on the host.

kernel(**inputs) takes the FULL inputs and returns the FULL [N, 512] output.
"""


import math
import ml_dtypes as _ml_dtypes
import numpy as np

import concourse.bacc as bacc
import concourse.bass as bass
import concourse.mybir as mybir
import concourse.tile as tile
from concourse.library_config import mlp as mlp_library

F32 = mybir.dt.float32
I16 = mybir.dt.int16
AX = mybir.AxisListType
OP = mybir.AluOpType
ACTF = mybir.ActivationFunctionType


# ----------------------------------------------------------------------------
# Host preprocessing
# ----------------------------------------------------------------------------

class Cfg:
    pass


def preprocess(x, edge_index, Ws, gammas, betas, C=8, G=2, eps=1e-5):
    """Build per-core device inputs + schedule constants from the graph."""
    N, D = x.shape
    assert D == 128
    L = Ws.shape[0]
    E = edge_index.shape[1]

    NPC = (N + C - 1) // C          # nodes per core (max)
    NT = (NPC + 127) // 128         # tiles per core
    NTP = NT * 128                  # slots per core
    SLOTS = C * NTP
    HALF = (C // 2) * NTP
    assert HALF < 32768 and (SLOTS - HALF) < 32768

    src = edge_index[0].astype(np.int64)
    dst = edge_index[1].astype(np.int64)

    deg = np.bincount(dst, minlength=N).astype(np.float64) + 1.0
    dinv = (1.0 / np.sqrt(deg)).astype(np.float32)

    # --- node -> core assignment: deal round-robin in degree order -------
    order = np.argsort(-deg, kind="stable")
    core_of = np.empty(N, np.int64)
    core_of[order] = np.arange(N) % C

    # --- per-node A/B in-degree (A = src owned by cores < C/2) -----------
    srcA = core_of[src] < (C // 2)
    selfA = core_of < (C // 2)
    dA = np.bincount(dst[srcA], minlength=N) + selfA.astype(np.int64)
    dB = np.bincount(dst[~srcA], minlength=N) + (~selfA).astype(np.int64)

    # --- per-core tile packing (greedy balance of dA+dB) -----------------
    tile_of = np.empty(N, np.int64)
    pos_of = np.empty(N, np.int64)
    maxA = 0
    maxB = 0
    for c in range(C):
        nodes = np.where(core_of == c)[0]
        nodes = nodes[np.argsort(-(dA[nodes] + dB[nodes]), kind="stable")]
        loadA = np.zeros(NT, np.float64)
        loadB = np.zeros(NT, np.float64)
        cnt = np.zeros(NT, np.int64)
        tA = max(dA[nodes].sum() / NT, 1.0)
        tB = max(dB[nodes].sum() / NT, 1.0)
        for v in nodes:
            # minimize the worse of the two normalized loads after adding v
            score = np.maximum((loadA + dA[v]) / tA, (loadB + dB[v]) / tB)
            score = score + np.where(cnt >= 128, 1e18, 0.0)
            t = int(np.argmin(score))
            tile_of[v] = t
            pos_of[v] = cnt[t]
            cnt[t] += 1
            loadA[t] += dA[v]
            loadB[t] += dB[v]
        # repair pass: push any tile over the 9-chunk budget (1152 edges)
        # back under it by moving its smallest nodes to underloaded tiles
        cap = 9 * 128
        tiles_of_core = {t: [] for t in range(NT)}
        for v in nodes:
            tiles_of_core[tile_of[v]].append(v)
        for _ in range(4000):
            worstA = int(np.argmax(loadA))
            worstB = int(np.argmax(loadB))
            if loadA[worstA] <= cap and loadB[worstB] <= cap:
                break
            side, worst = ((0, worstA) if loadA[worstA] - cap
                           >= loadB[worstB] - cap else (1, worstB))
            moved = False
            dd = dA if side == 0 else dB
            cand = sorted((v for v in tiles_of_core[worst] if dd[v] > 0),
                          key=lambda v: -dd[v])
            for v in cand:
                dst_ts = np.where(
                    (cnt < 128)
                    & (loadA + dA[v] <= cap)
                    & (loadB + dB[v] <= cap))[0]
                dst_ts = [t for t in dst_ts if t != worst]
                if dst_ts:
                    t2 = min(dst_ts, key=lambda t: loadA[t] + loadB[t])
                    tiles_of_core[worst].remove(v)
                    tiles_of_core[t2].append(v)
                    tile_of[v] = t2
                    loadA[worst] -= dA[v]; loadB[worst] -= dB[v]
                    loadA[t2] += dA[v]; loadB[t2] += dB[v]
                    cnt[worst] -= 1; cnt[t2] += 1
                    moved = True
                    break
            if not moved:
                break
        # reassign positions after moves
        for t in range(NT):
            for i, v in enumerate(tiles_of_core[t]):
                pos_of[v] = i
        maxA = max(maxA, int(loadA.max()))
        maxB = max(maxB, int(loadB.max()))

    LA = (maxA + 127) // 128
    LB = (maxB + 127) // 128
    CH = LA + LB

    slot_of = core_of * NTP + tile_of * 128 + pos_of
    node_of_slot = np.full(SLOTS, -1, np.int64)
    node_of_slot[slot_of] = np.arange(N)

    # --- edge arrays (with self-loops), assigned to dst (core,tile) ------
    e_src = np.concatenate([src, np.arange(N)])
    e_dst = np.concatenate([dst, np.arange(N)])
    e_srcslot = slot_of[e_src]
    e_grpB = (e_srcslot >= HALF).astype(np.int64)
    e_core = core_of[e_dst]
    e_tile = tile_of[e_dst]
    e_pos = pos_of[e_dst]

    key = ((e_core * NT) + e_tile) * 2 + e_grpB
    eorder = np.argsort(key, kind="stable")
    key_s = key[eorder]
    srcslot_s = e_srcslot[eorder]
    pos_s = e_pos[eorder]
    # rank within each (core,tile,grp) segment
    seg_start = np.zeros(len(key_s), np.int64)
    first = np.r_[True, key_s[1:] != key_s[:-1]]
    seg_ids = np.cumsum(first) - 1
    starts = np.where(first)[0]
    rank = np.arange(len(key_s)) - starts[seg_ids]

    # idx / dstb tables: per core, chunk-slot grid [NT, CH, 128]
    idx_grid = np.zeros((C, NT, CH, 128), np.int64)      # gather row index
    dstb_grid = np.full((C, NT, CH, 128), -1.0, np.float32)  # dst pos or -1

    g_core = key_s // (NT * 2)
    g_tile = (key_s // 2) % NT
    g_grp = key_s % 2
    chunk = np.where(g_grp == 0, rank // 128, LA + rank // 128)
    p = rank % 128
    assert np.all(np.where(g_grp == 0, chunk < LA, chunk < CH))
    idxval = np.where(g_grp == 0, srcslot_s, srcslot_s - HALF)
    idx_grid[g_core, g_tile, chunk, p] = idxval
    dstb_grid[g_core, g_tile, chunk, p] = pos_s.astype(np.float32)

    # --- per-core gather-call index stream (call-major, wrapped [16,*]) --
    groups = [list(range(g, min(g + G, NT))) for g in range(0, NT, G)]
    call_plan = []  # (grp, tiles, col_off_in_idx16, num_idxs)
    idx16 = np.zeros((C, NT * CH * 128), np.int64)
    off = 0
    for tiles_g in groups:
        for grp, lo, hi in ((0, 0, LA), (1, LA, CH)):
            n = len(tiles_g) * (hi - lo) * 128
            seq = idx_grid[:, tiles_g, lo:hi, :]  # [C, Gt, LX, 128]
            # call's output column block (ti*LX + cx) holds slot p=j%128,
            # unwrapped j = (ti*LX+cx)*128 + p
            seq = seq.transpose(0, 1, 2, 3).reshape(C, n)
            idx16[:, off:off + n] = seq
            call_plan.append((grp, tuple(tiles_g), off, n))
            off += n
    assert off == NT * CH * 128
    # wrapped layout: element j -> [j%16, j//16], tiled to 128 partitions
    idx16w = idx16.reshape(C, -1, 16).transpose(0, 2, 1).astype(np.int16)
    idx16w = np.tile(idx16w, (1, 8, 1))  # [C, 128, NT*CH*8]

    dinv_slot = np.zeros(SLOTS, np.float32)
    dinv_slot[slot_of] = dinv

    per_core = []
    for c in range(C):
        sl = slice(c * NTP, (c + 1) * NTP)
        x_fm = np.zeros((128, NTP), np.float32)
        vs = slot_of[core_of == c] - c * NTP
        x_fm[:, vs] = x[core_of == c].T
        d = {
            "x_fm": x_fm,
            "dinv_fm": np.tile(dinv_slot[sl][None, :], (128, 1)),
            "dinv_nm": dinv_slot[sl].reshape(NT, 128).T.copy(),
            "idx16": idx16w[c],
            "dstb": dstb_grid[c].reshape(NT * CH, 128).T.astype(
                _ml_dtypes.bfloat16),
            "iota": np.tile(np.arange(128, dtype=np.float32)[None, :],
                            (128, 1)).astype(_ml_dtypes.bfloat16),
            "Ws": Ws.astype(np.float32),
            "gammaT": gammas.T.astype(np.float32).copy(),
            "betaT": betas.T.astype(np.float32).copy(),
        }
        per_core.append(d)

    cfg = Cfg()
    cfg.N, cfg.D, cfg.L, cfg.C, cfg.E = N, D, L, C, E
    cfg.NPC, cfg.NT, cfg.NTP, cfg.SLOTS, cfg.HALF = NPC, NT, NTP, SLOTS, HALF
    cfg.LA, cfg.LB, cfg.CH, cfg.G = LA, LB, CH, G
    cfg.groups = groups
    cfg.call_plan = call_plan
    cfg.eps = eps
    cfg.core_of = core_of
    cfg.slot_of = slot_of
    cfg.node_of_slot = node_of_slot
    return cfg, per_core


def assemble_output(cfg, x, core_outs):
    """core_outs: list of o_out arrays [L,128,NTP] per core -> [N, (L+1)*128]."""
    N, L, C, NTP = cfg.N, cfg.L, cfg.C, cfg.NTP
    out = np.empty((N, (L + 1) * 128), np.float32)
    out[:, :128] = x
    for c in range(C):
        slots = cfg.node_of_slot[c * NTP:(c + 1) * NTP]
        valid = slots >= 0
        nodes = slots[valid]
        for l in range(L):
            out[nodes, (l + 1) * 128:(l + 2) * 128] = core_outs[c][l][:, valid].T
    return out


# ----------------------------------------------------------------------------
# Bass kernel
# ----------------------------------------------------------------------------

def build_nc(cfg, reps=1, skip=()):
    NT, NTP, CH, LA, LB = cfg.NT, cfg.NTP, cfg.CH, cfg.LA, cfg.LB
    SLOTS, HALF, L, C = cfg.SLOTS, cfg.HALF, cfg.L, cfg.C
    IDXW = NT * CH * 8

    nc = bacc.Bacc("TRN2", target_bir_lowering=False, num_devices=C)

    x_fm_t = nc.dram_tensor("x_fm", [128, NTP], F32, kind="ExternalInput")
    dinv_fm_t = nc.dram_tensor("dinv_fm", [128, NTP], F32, kind="ExternalInput")
    dinv_nm_t = nc.dram_tensor("dinv_nm", [128, NT], F32, kind="ExternalInput")
    idx16_t = nc.dram_tensor("idx16", [128, IDXW], I16, kind="ExternalInput")
    dstb_t = nc.dram_tensor("dstb", [128, NT * CH], mybir.dt.bfloat16,
                            kind="ExternalInput")
    iota_t = nc.dram_tensor("iota", [128, 128], mybir.dt.bfloat16,
                            kind="ExternalInput")
    Ws_t = nc.dram_tensor("Ws", [L, 128, 128], F32, kind="ExternalInput")
    gammaT_t = nc.dram_tensor("gammaT", [128, L], F32, kind="ExternalInput")
    betaT_t = nc.dram_tensor("betaT", [128, L], F32, kind="ExternalInput")
    o_out_t = nc.dram_tensor("o_out", [L, 128, NTP], F32, kind="ExternalOutput")

    with tile.TileContext(nc) as tc:
        with (
            tc.tile_pool(name="persist", bufs=1) as pp,
            tc.tile_pool(name="gath", bufs=2) as gp,
            tc.tile_pool(name="work", bufs=2) as wp,
            tc.tile_pool(name="psum", bufs=2, space="PSUM") as psp,
            tc.tile_pool(name="dram", bufs=1, space="DRAM") as dp,
        ):
            o_fm = pp.tile([128, NTP], F32)
            m_sb = pp.tile([128, NT, 128], mybir.dt.bfloat16)
            dinv_fm = pp.tile([128, NTP], F32)
            dinv_nm = pp.tile([128, NT], F32)
            idx16 = pp.tile([128, IDXW], I16)
            dstb = pp.tile([128, NT * CH], mybir.dt.bfloat16)
            iota = pp.tile([128, 128], mybir.dt.bfloat16)
            Wt = pp.tile([128, L, 128], F32)
            gammaT = pp.tile([128, L], F32)
            betaT = pp.tile([128, L], F32)
            stat = pp.tile([128, 2], F32)
            statr = pp.tile([128, 2], F32)
            statg = pp.tile([128, 2, 8], F32)
            prm = pp.tile([128, 8], F32)

            m_slice_ds = [dp.tile([NTP, 128], mybir.dt.bfloat16,
                                  name=f"m_slice_{l}")
                          for l in range(L * reps)]
            m_full_ds = [dp.tile([SLOTS, 128], mybir.dt.bfloat16,
                                 addr_space="Shared",
                                 name=f"m_full_{l}") for l in range(L * reps)]
            stat_in_ds = [dp.tile([128, 2], F32, name=f"stat_in_{l}")
                          for l in range(L * reps)]
            stat_out_ds = [dp.tile([128 * 8, 2], F32, addr_space="Shared",
                                   name=f"stat_out_{l}") for l in range(L * reps)]

            # --- load phase ---------------------------------------------
            nc.gpsimd.load_library(mlp_library)
            nc.sync.dma_start(o_fm[:], x_fm_t[:])
            nc.sync.dma_start(dinv_fm[:], dinv_fm_t[:])
            nc.sync.dma_start(dinv_nm[:], dinv_nm_t[:])
            nc.sync.dma_start(idx16[:], idx16_t[:])
            nc.sync.dma_start(dstb[:], dstb_t[:])
            nc.sync.dma_start(iota[:], iota_t[:])
            nc.sync.dma_start(Wt[:], Ws_t[:].rearrange("l k f -> k l f"))
            nc.sync.dma_start(gammaT[:], gammaT_t[:])
            nc.sync.dma_start(betaT[:], betaT_t[:])

            inv_n = 1.0 / float(cfg.N)

            for rep in range(reps):
              if rep > 0:
                nc.sync.dma_start(o_fm[:], x_fm_t[:])
              for l in range(L):
                m_slice_d = m_slice_ds[rep * L + l]
                m_full_d = m_full_ds[rep * L + l]
                stat_in_d = stat_in_ds[rep * L + l]
                stat_out_d = stat_out_ds[rep * L + l]
                # --- m = (o @ W_l) * dinv  (node-major blocks) ----------
                for b in range(NT):
                    pm = psp.tile([128, 128], F32, name="pm")
                    nc.tensor.matmul(
                        pm[:], lhsT=o_fm[:, b * 128:(b + 1) * 128],
                        rhs=Wt[:, l, :], start=True, stop=True)
                    nc.scalar.activation(
                        m_sb[:, b, :], pm[:], ACTF.Copy,
                        scale=dinv_nm[:, b:b + 1])
                # m -> DRAM (node-major rows), then all-gather
                nc.sync.dma_start(
                    m_slice_d[:].rearrange("(b p) f -> p b f", p=128), m_sb[:])
                if "ag" not in skip:
                    nc.gpsimd.collective_compute(
                        "AllGather", OP.bypass,
                        replica_groups=[list(range(C))],
                        ins=[m_slice_d[:]], outs=[m_full_d[:]])

                # --- gather + aggregate ---------------------------------
                ci = 0
                for tiles_g in cfg.groups:
                    gt = len(tiles_g)
                    grpA, _, offA, nA = cfg.call_plan[ci]
                    grpB_, _, offB, nB = cfg.call_plan[ci + 1]
                    ci += 2
                    gbufA = gp.tile([128, gt * LA, 128], mybir.dt.bfloat16,
                                    name="gbufA")
                    gbufB = gp.tile([128, gt * LB, 128], mybir.dt.bfloat16,
                                    name="gbufB")
                    if "gather" in skip:
                        nc.vector.memset(gbufA[:], 0.0)
                        nc.vector.memset(gbufB[:], 0.0)
                    else:
                        nc.gpsimd.dma_gather(
                            gbufA[:], m_full_d[0:HALF, :],
                            idx16[:, offA // 16:(offA + nA) // 16],
                            nA, nA, 128, single_packet=False)
                        nc.gpsimd.dma_gather(
                            gbufB[:], m_full_d[HALF:SLOTS, :],
                            idx16[:, offB // 16:(offB + nB) // 16],
                            nB, nB, 128, single_packet=False)
                    for ti, t in enumerate(tiles_g):
                        S = wp.tile([128, CH, 128], mybir.dt.bfloat16,
                                    name="S")
                        nc.vector.tensor_tensor(
                            S[:],
                            dstb[:, t * CH:(t + 1) * CH].unsqueeze(2)
                                .to_broadcast([128, CH, 128]),
                            iota[:].unsqueeze(1).to_broadcast([128, CH, 128]),
                            OP.is_equal)
                        pa = psp.tile([128, 128], F32, name="pa")
                        if "aggmm" not in skip:
                          chlist = range(CH) if "ch4" not in skip else [0, 1, LA, LA+1]
                          for c in chlist:
                            if c < LA:
                                g = gbufA[:, ti * LA + c, :]
                            else:
                                g = gbufB[:, ti * LB + (c - LA), :]
                            nc.tensor.matmul(
                                pa[:], lhsT=g, rhs=S[:, c, :],
                                start=(c == 0), stop=(c == CH - 1))
                        elif False:
                          pass
                        else:
                          nc.tensor.matmul(
                                pa[:], lhsT=gbufA[:, 0, :], rhs=S[:, 0, :],
                                start=True, stop=True)
                        # drain + dinv_dst scale (feature-major)
                        nc.vector.tensor_tensor(
                            o_fm[:, t * 128:(t + 1) * 128], pa[:],
                            dinv_fm[:, t * 128:(t + 1) * 128], OP.mult)

                # --- BN stats (biased, over all N real nodes) -----------
                nc.vector.tensor_reduce(
                    stat[:, 0:1], o_fm[:], axis=AX.X, op=OP.add)
                nc.scalar.square(m_sb[:].rearrange("p b f -> p (b f)"), o_fm[:])
                nc.vector.tensor_reduce(
                    stat[:, 1:2], m_sb[:].rearrange("p b f -> p (b f)"),
                    axis=AX.X, op=OP.add)
                nc.sync.dma_start(stat_in_d[:], stat[:])
                if "ar" not in skip:
                    nc.gpsimd.collective_compute(
                        "AllGather", OP.bypass,
                        replica_groups=[list(range(C))],
                        ins=[stat_in_d[:]], outs=[stat_out_d[:]])
                nc.sync.dma_start(
                    statg[:],
                    stat_out_d[:].rearrange("(r p) j -> p j r", p=128))
                nc.vector.tensor_tensor(statg[:, :, 0:4], statg[:, :, 0:4],
                                        statg[:, :, 4:8], OP.add)
                nc.vector.tensor_tensor(statg[:, :, 0:2], statg[:, :, 0:2],
                                        statg[:, :, 2:4], OP.add)
                nc.vector.tensor_tensor(statr[:, 0:1], statg[:, 0, 0:1],
                                        statg[:, 0, 1:2], OP.add)
                nc.vector.tensor_tensor(statr[:, 1:2], statg[:, 1, 0:1],
                                        statg[:, 1, 1:2], OP.add)

                # mu = S1/N; var = S2/N - mu^2; a = gamma*rsqrt(var+eps);
                # b = beta - mu*a
                mu = prm[:, 0:1]
                msq = prm[:, 1:2]
                var = prm[:, 2:3]
                rsd = prm[:, 3:4]
                a_ = prm[:, 4:5]
                b_ = prm[:, 5:6]
                nc.vector.tensor_scalar(
                    out=prm[:, 0:2], in0=statr[:], scalar1=inv_n, scalar2=None,
                    op0=OP.mult)
                nc.vector.tensor_tensor(var, mu, mu, OP.mult)
                nc.vector.tensor_tensor(var, msq, var, OP.subtract)
                nc.vector.tensor_scalar(
                    out=var, in0=var, scalar1=float(cfg.eps), scalar2=None,
                    op0=OP.add)
                nc.vector.reciprocal(rsd, var)
                nc.scalar.sqrt(rsd, rsd)
                nc.vector.tensor_tensor(a_, rsd, gammaT[:, l:l + 1], OP.mult)
                nc.vector.tensor_tensor(b_, mu, a_, OP.mult)
                nc.vector.tensor_tensor(b_, betaT[:, l:l + 1], b_, OP.subtract)

                # o = relu(a*agg + b), in place
                nc.scalar.activation(
                    o_fm[:], o_fm[:], ACTF.Relu, bias=b_, scale=a_)

                nc.sync.dma_start(o_out_t[l], o_fm[:])

    nc.compile()
    return nc


# ----------------------------------------------------------------------------
# Entry point
# ----------------------------------------------------------------------------

_CACHE = {}


def kernel(x, edge_index, Ws, bs, gammas, betas):
    import numpy as _np
    from concourse.bass_utils import run_bass_kernel_spmd

    x = _np.asarray(x, dtype=_np.float32)
    edge_index = _np.asarray(edge_index)
    Ws = _np.asarray(Ws, dtype=_np.float32)
    gammas = _np.asarray(gammas, dtype=_np.float32)
    betas = _np.asarray(betas, dtype=_np.float32)

    cfg, per_core = preprocess(x, edge_index, Ws, gammas, betas, C=8)
    key = (cfg.NT, cfg.LA, cfg.LB, tuple(map(tuple, cfg.groups)))
    if key not in _CACHE:
        _CACHE[key] = build_nc(cfg)
    nc = _CACHE[key]
    in_maps = [{k: _np.ascontiguousarray(v) for k, v in d.items()}
               for d in per_core]
    res = run_bass_kernel_spmd(nc, in_maps, core_ids=list(range(cfg.C)))
    core_outs = [res.results[c]["o_out"].reshape(cfg.L, 128, cfg.NTP)
                 for c in range(cfg.C)]
    return assemble_output(cfg, x, core_outs)

